# revision 33
# baseline (speedup 1.0000x reference)
"""Trainium2 Bass kernel for MinimalLBS (B=32, T=128, N=2048, J=52, Jb=21, L=16).

Strategy: data-parallel over B across 8 NeuronCores (4 samples per core).
Host does layout-only prep (transposes / bf16 casts / augmented-contraction
rows); all FLOPs (Rodrigues, blend/pose matmuls, skinning, per-vertex matvec)
run on device.

Device math per sample b (all matmul inputs bf16, PSUM accumulation f32):
  1. Rodrigues on DVE/ACT from pose [t,63] -> pose_feature pf [t,189] (bf16),
     PE-transposed to pfT [189,t].
  2. v_posed directly in vertex-major layout: for each n-chunk (128) and
     component c: vps[n,c,t] = sum_p pd_c[p,n]*pfT[p,t] (K split 128+61)
                             + sum_l sd_c[l,n]*betasT[l,t]   (K=17, row 16 is
                               v_template via ones row in betasT)
  3. Skinning: ts[n,i,j,t] = sum_k wT[k,n]*ar[k,i,j,t], K=53 where row 52 is
     (ones x trans[t,i] at j=3) so translation is folded in. Only i<3 kept.
  4. Matvec (DVE/GpSimd): sens[n,i,t] = sum_{j<3} ts[n,i,j,t]*v3[n,j,t]
                                        + ts[n,i,3,t]
  5. DMA out as [n,i,t] (contiguous rows); host reassembles to (B,T,N,3).
"""

import sys

sys.path.insert(0, "/opt/trn_rl_repo")

import math

import ml_dtypes
import numpy as np

import concourse.bacc as bacc
import concourse.bass as bass
import concourse.mybir as mybir
import concourse.tile as tile
from concourse import bass_utils, masks

F32 = mybir.dt.float32
BF16 = mybir.dt.bfloat16
NPBF16 = ml_dtypes.bfloat16

B, T, N, JB, J, L = 32, 128, 2048, 21, 52, 16
NCORES = 8
NB = B // NCORES          # samples per core
PF = JB * 9               # 189 pose-feature dims
KB0, KB1 = 128, PF - 128  # K split for the pose matmul
JA = J + 1                # joints + translation row
LA = L + 1                # betas + template row
NCH = N // 128            # n-chunks per sample

_CACHED = {}


def _build_nc(taps=False):
    nc = bacc.Bacc("TRN2", target_bir_lowering=False, debug=False)

    pose_d = nc.dram_tensor("pose", [T, NB, JB, 3], F32, kind="ExternalInput")
    pd_d = nc.dram_tensor("pd", [NB, 3, 2, 128, N], BF16, kind="ExternalInput")
    sdt_d = nc.dram_tensor("sdt", [NB, LA, 3, N], BF16, kind="ExternalInput")
    betat_d = nc.dram_tensor("betat", [NB, LA, T], BF16, kind="ExternalInput")
    wt_d = nc.dram_tensor("wt", [NB, JA, N], BF16, kind="ExternalInput")
    ar_d = nc.dram_tensor("ar", [NB, JA, 3, 4, T], BF16, kind="ExternalInput")
    out_d = nc.dram_tensor("out", [NB, NCH, 128, 3, T], BF16, kind="ExternalOutput")
    if taps:
        pf_t = nc.dram_tensor("pf_t", [T, NB, JB, 9], F32, kind="ExternalOutput")
        v3_t = nc.dram_tensor("v3_t", [NB, NCH, 128, 3, T], F32,
                              kind="ExternalOutput")
        ts_t = nc.dram_tensor("ts_t", [NB, NCH, 128, 3, 4, T], F32,
                              kind="ExternalOutput")
        pft_t = nc.dram_tensor("pft_t", [NB, 128, 2, T], F32,
                               kind="ExternalOutput")

    with tile.TileContext(nc) as tc:
        with (
            tc.tile_pool(name="const", bufs=1) as p_const,
            tc.tile_pool(name="rod", bufs=1) as p_rod,
            tc.tile_pool(name="pft", bufs=2) as p_pft,
            tc.tile_pool(name="big", bufs=2) as p_big,
            tc.tile_pool(name="small", bufs=2) as p_small,
            tc.tile_pool(name="mv", bufs=10) as p_mv,
            tc.tile_pool(name="psv", bufs=2, space="PSUM") as ps_v,
            tc.tile_pool(name="psts", bufs=2, space="PSUM") as ps_ts,
        ):
            ident = p_const.tile([128, 128], BF16)
            masks.make_identity(nc, ident[:])
            cst = p_const.tile([128, 2], F32)
            nc.vector.memset(cst[:, 0:1], math.pi / 2)
            nc.vector.memset(cst[:, 1:2], 1.0)

            # ---- Rodrigues for all NB samples at once: pose [t, nb, jb, 3]
            po = p_rod.tile([T, NB, JB, 3], F32)
            nc.sync.dma_start(po[:], pose_d[:])
            sq = p_rod.tile([T, NB, JB, 3], F32)
            nc.vector.tensor_tensor(sq[:], po[:], po[:], mybir.AluOpType.mult)
            a2 = p_rod.tile([T, NB, JB], F32)
            nc.vector.tensor_tensor(
                a2[:], sq[:, :, :, 0], sq[:, :, :, 1], mybir.AluOpType.add
            )
            a2b = p_rod.tile([T, NB, JB], F32)
            nc.vector.tensor_tensor(a2b[:], a2[:], sq[:, :, :, 2], mybir.AluOpType.add)
            a2c = p_rod.tile([T, NB, JB], F32)
            nc.vector.tensor_scalar_max(a2c[:], a2b[:], 1e-16)
            ang = p_rod.tile([T, NB, JB], F32)
            nc.scalar.sqrt(ang[:], a2c[:])
            inv = p_rod.tile([T, NB, JB], F32)
            nc.vector.reciprocal(inv[:], ang[:])
            s = p_rod.tile([T, NB, JB], F32)
            nc.scalar.activation(s[:], ang[:], mybir.ActivationFunctionType.Sin)
            co = p_rod.tile([T, NB, JB], F32)
            nc.scalar.activation(
                co[:], ang[:], mybir.ActivationFunctionType.Sin, bias=cst[:, 0:1]
            )
            u = p_rod.tile([T, NB, JB], F32)
            nc.scalar.activation(
                u[:], co[:], mybir.ActivationFunctionType.Identity,
                bias=cst[:, 1:2], scale=-1.0,
            )
            ax = p_rod.tile([T, NB, JB, 3], F32)
            nc.vector.tensor_tensor(
                ax[:], po[:], inv[:].unsqueeze(3).broadcast_to((T, NB, JB, 3)),
                mybir.AluOpType.mult,
            )

            pf = p_rod.tile([T, NB, JB, 9], BF16)

            def axc(i):
                return ax[:, :, :, i]

            prods = {}
            for (a, b2), nm in [
                ((0, 0), "xx"), ((1, 1), "yy"), ((2, 2), "zz"),
                ((0, 1), "xy"), ((0, 2), "xz"), ((1, 2), "yz"),
            ]:
                t_ = p_rod.tile([T, NB, JB], F32, tag=f"pr_{nm}")
                nc.gpsimd.tensor_tensor(t_[:], axc(a), axc(b2), mybir.AluOpType.mult)
                prods[nm] = t_
            qs = {}
            for i, nm in [(0, "qx"), (1, "qy"), (2, "qz")]:
                t_ = p_rod.tile([T, NB, JB], F32, tag=f"q_{nm}")
                nc.gpsimd.tensor_tensor(t_[:], s[:], axc(i), mybir.AluOpType.mult)
                qs[nm] = t_
            os_ = {}
            for nm in ["xy", "xz", "yz"]:
                t_ = p_rod.tile([T, NB, JB], F32, tag=f"o_{nm}")
                nc.gpsimd.tensor_tensor(
                    t_[:], u[:], prods[nm][:], mybir.AluOpType.mult
                )
                os_[nm] = t_
            for di, nm in [(0, "xx"), (4, "yy"), (8, "zz")]:
                d_ = p_rod.tile([T, NB, JB], F32, tag=f"d_{nm}")
                nc.vector.tensor_scalar_add(d_[:], prods[nm][:], -1.0)
                nc.vector.tensor_tensor(
                    pf[:, :, :, di], u[:], d_[:], mybir.AluOpType.mult
                )
            for e, o_nm, q_nm, op in [
                (1, "xy", "qz", mybir.AluOpType.subtract),
                (3, "xy", "qz", mybir.AluOpType.add),
                (2, "xz", "qy", mybir.AluOpType.add),
                (6, "xz", "qy", mybir.AluOpType.subtract),
                (5, "yz", "qx", mybir.AluOpType.subtract),
                (7, "yz", "qx", mybir.AluOpType.add),
            ]:
                nc.vector.tensor_tensor(
                    pf[:, :, :, e], os_[o_nm][:], qs[q_nm][:], op
                )

            # ---- per-sample pipeline
            for nb in range(NB):
                # pfT [128, 2, T]: K-blocks of transposed pose features
                pft = p_pft.tile([128, 2, T], BF16, tag="pft")
                nc.vector.memset(pft[:], 0.0)
                pf_nb = pf[:, nb].rearrange("t j e -> t (j e)")
                tp0 = ps_v.tile([128, 3, T], BF16, tag="psv")
                nc.tensor.transpose(tp0[:, 0, :], pf_nb[:, 0:128], ident[:])
                nc.scalar.copy(pft[:, 0, :], tp0[:, 0, :])
                tp1 = ps_v.tile([128, 3, T], BF16, tag="psv")
                nc.tensor.transpose(tp1[0:KB1, 0, :], pf_nb[:, 128:PF], ident[:])
                nc.scalar.copy(pft[0:KB1, 1, :], tp1[0:KB1, 0, :])
                if taps:
                    pftc = p_pft.tile([128, 2, T], F32, tag="pftc")
                    nc.vector.tensor_copy(pftc[:], pft[:])
                    nc.sync.dma_start(pft_t[nb], pftc[:])

                pd_s = p_big.tile([128, 3, 2, N], BF16, tag="pd")
                nc.sync.dma_start(
                    pd_s[:], pd_d[nb].rearrange("c e k n -> k c e n")
                )
                sdt_s = p_small.tile([LA, 3, N], BF16, tag="sdt")
                nc.sync.dma_start(sdt_s[:], sdt_d[nb])
                betat_s = p_small.tile([LA, T], BF16, tag="betat")
                nc.sync.dma_start(betat_s[:], betat_d[nb])
                wt_s = p_small.tile([JA, N], BF16, tag="wt")
                nc.sync.dma_start(wt_s[:], wt_d[nb])
                ar_s = p_small.tile([JA, 3, 4, T], BF16, tag="ar")
                nc.sync.dma_start(ar_s[:], ar_d[nb])

                for nch in range(NCH):
                    n0 = nch * 128
                    # v_posed [n, c, t]
                    vps = ps_v.tile([128, 3, T], F32, tag="psv")
                    for c in range(3):
                        nc.tensor.matmul(
                            vps[:, c, :],
                            pd_s[:, c, 0, n0 : n0 + 128],
                            pft[:, 0, :],
                            start=True,
                            stop=False,
                        )
                        nc.tensor.matmul(
                            vps[:, c, :],
                            pd_s[:, c, 1, n0 : n0 + 128],
                            pft[:, 1, :],
                            start=False,
                            stop=False,
                        )
                        nc.tensor.matmul(
                            vps[:, c, :],
                            sdt_s[:, c, n0 : n0 + 128],
                            betat_s[:],
                            start=False,
                            stop=True,
                        )
                    v3 = p_mv.tile([128, 3, T], BF16, tag="v3")
                    nc.scalar.copy(v3[:], vps[:])

                    # skinning ts [n, i, j, t]
                    ts = ps_ts.tile([128, 3, 4, T], F32, tag="psts")
                    for i in range(3):
                        nc.tensor.matmul(
                            ts[:, i],
                            wt_s[:, n0 : n0 + 128],
                            ar_s[:, i].rearrange("k j t -> k (j t)"),
                            start=True,
                            stop=True,
                        )

                    # matvec. For half the chunks ACT (which has slack)
                    # evacuates ts to bf16 SBUF so the DVE multiply runs in
                    # 2x mode; the other half multiplies straight from PSUM
                    # at 1x. This balances DVE ~= ACT.
                    if nch % 2 == 0:
                        tsb = p_mv.tile([128, 3, 4, T], BF16, tag="tsb")
                        nc.scalar.copy(tsb[:], ts[:])
                        mul_src = tsb[:, :, 0:3, :]
                        t3_src = tsb[:, :, 3, :]
                    else:
                        t3 = p_mv.tile([128, 3, T], BF16, tag="t3")
                        nc.scalar.copy(t3[:], ts[:, :, 3, :])
                        mul_src = ts[:, :, 0:3, :]
                        t3_src = t3[:]
                    pm = p_mv.tile([128, 3, 3, T], BF16, tag="pm")
                    nc.vector.tensor_tensor(
                        pm[:],
                        mul_src,
                        v3[:].unsqueeze(1).broadcast_to((128, 3, 3, T)),
                        mybir.AluOpType.mult,
                    )
                    s1 = p_mv.tile([128, 3, T], BF16, tag="s1")
                    nc.vector.tensor_tensor(
                        s1[:], pm[:, :, 0, :], pm[:, :, 1, :], mybir.AluOpType.add
                    )
                    s2 = p_mv.tile([128, 3, T], BF16, tag="s2")
                    nc.gpsimd.tensor_tensor(
                        s2[:], s1[:], pm[:, :, 2, :], mybir.AluOpType.add
                    )
                    sens = p_mv.tile([128, 3, T], BF16, tag="sens")
                    nc.vector.tensor_tensor(
                        sens[:], s2[:], t3_src, mybir.AluOpType.add
                    )
                    nc.sync.dma_start(out_d[nb, nch], sens[:])
                    if taps:
                        nc.sync.dma_start(v3_t[nb, nch], v3[:])
                        tsc = p_mv.tile([128, 3, 4, T], F32, tag="tsc")
                        nc.scalar.copy(tsc[:], ts[:])
                        nc.sync.dma_start(ts_t[nb, nch], tsc[:])
            if taps:
                pfc = p_rod.tile([T, NB, JB, 9], F32, tag="pfc")
                nc.vector.tensor_copy(pfc[:], pf[:])
                nc.sync.dma_start(pf_t[:], pfc[:])

    nc.compile()
    return nc


def _prep_core(c, pose_body, trans, betas, A, v_template, shapedirs, posedirs,
               lbs_weights):
    bs = slice(NB * c, NB * (c + 1))
    pose = np.ascontiguousarray(
        pose_body[bs].transpose(1, 0, 2).reshape(T, NB, JB, 3)
    ).astype(np.float32)

    pdc = posedirs[bs].reshape(NB, PF, N, 3)           # [nb, p, n, c]
    pd = np.zeros((NB, 3, 2, 128, N), dtype=NPBF16)
    pd_t = pdc.transpose(0, 3, 1, 2)                   # [nb, c, p, n]
    pd[:, :, 0, :, :] = pd_t[:, :, 0:128].astype(NPBF16)
    pd[:, :, 1, 0:KB1, :] = pd_t[:, :, 128:PF].astype(NPBF16)

    sdt = np.empty((NB, LA, 3, N), dtype=NPBF16)
    sdt[:, 0:L, :, :] = shapedirs[bs].transpose(0, 3, 2, 1).astype(NPBF16)
    sdt[:, L, :, :] = v_template[bs].transpose(0, 2, 1).astype(NPBF16)

    betat = np.empty((NB, LA, T), dtype=NPBF16)
    betat[:, 0:L, :] = betas[bs].transpose(0, 2, 1).astype(NPBF16)
    betat[:, L, :] = np.ones((NB, T), dtype=NPBF16)

    wt = np.empty((NB, JA, N), dtype=NPBF16)
    wt[:, 0:J, :] = lbs_weights[bs].transpose(0, 2, 1).astype(NPBF16)
    wt[:, J, :] = np.ones((NB, N), dtype=NPBF16)

    ar = np.zeros((NB, JA, 3, 4, T), dtype=NPBF16)
    ar[:, 0:J] = A[bs, :, :, 0:3, :].transpose(0, 2, 3, 4, 1).astype(NPBF16)
    ar[:, J, :, 3, :] = trans[bs].transpose(0, 2, 1).astype(NPBF16)

    return {
        "pose": pose, "pd": pd, "sdt": sdt, "betat": betat, "wt": wt, "ar": ar,
    }


def kernel(pose_body, trans, betas, A, v_template, shapedirs, posedirs,
           lbs_weights):
    if "nc" not in _CACHED:
        _CACHED["nc"] = _build_nc()
    nc = _CACHED["nc"]

    args = (pose_body, trans, betas, A, v_template, shapedirs, posedirs,
            lbs_weights)
    args = tuple(np.asarray(a, dtype=np.float32) for a in args)
    in_maps = [_prep_core(c, *args) for c in range(NCORES)]

    res = bass_utils.run_bass_kernel_spmd(nc, in_maps, core_ids=list(range(NCORES)))

    # out [NB, NCH, 128, 3, T] per core -> (B, T, N, 3)
    full = np.stack(
        [res.results[c]["out"].astype(np.float32) for c in range(NCORES)]
    )
    full = full.reshape(B, NCH, 128, 3, T).transpose(0, 4, 1, 2, 3)
    return np.ascontiguousarray(full.reshape(B, T, N, 3).astype(np.float32))
